# revision 15
# baseline (speedup 1.0000x reference)
"""Cross-attention kernel for Trainium2, 8-core tensor-parallel over heads.

Problem (fixed shapes, fp32 in/out):
    patch_embed [2, 2048, 1024], pixel_embed [2, 2048, 1024]
    Wq/Wk/Wv [1024, 1024], Wo [1024, 1024], bo [1024]
    16 heads x 64 dim_head, softmax cross-attention, out [2, 2048, 1024].

Sharding: core c handles batch b = c // 4 and head-group g = c % 4
(4 heads = 256 inner cols). Each core computes a partial output
(its heads' contribution to out @ Wo); host sums the 4 partials per
batch and adds the bias.

v2 design (bf16 matmuls, host-side transpose):
  - Host pre-transposes patch/pixel to [d, seq] and casts everything to
    bf16, so the device needs no transposes at all (rel err ~6e-3 vs
    the 2e-2 gate; logits stay in fp32 PSUM).
  - Projections run kt-outer across 8 PSUM banks so the matmuls
    pipeline with the input DMA chunks.
  - Attention: per (qc, pair) accumulate po[65, 512] (row 64 = Z via a
    ones-column in V); exp on ACT from fp32 PSUM straight to bf16 SBUF.
  - Normalization (DVE reciprocal + gpsimd broadcast + DVE mul) is
    pipelined behind the PE via 3 po buffers.
  - Output projection for query chunk qc is emitted mid-way through
    attention chunk qc+1 so its matmuls never head-of-line-block the PE.
"""

import numpy as np

HEADS = 16
DH = 64
B = 2
N = 2048          # query seq len
M = 2048          # key seq len
D = 1024
N_CORES = 8
HPC = 4           # heads per core
C = HPC * DH      # 256 inner cols per core
SCALE = DH ** -0.5
P = 128
FREE = 512        # fp32 PSUM bank free dim
KT_D = D // P     # 8 contraction tiles for projections
JT = M // P       # 16 key tiles
QC = N // FREE    # 4 query chunks

_cache = {}


def _build_nc():
    import concourse.bacc as bacc
    import concourse.mybir as mybir
    import concourse.tile as tile

    F32 = mybir.dt.float32
    BF16 = mybir.dt.bfloat16
    EXP = mybir.ActivationFunctionType.Exp

    nc = bacc.Bacc("TRN2", target_bir_lowering=False, debug=False,
                   num_devices=N_CORES)

    xt = nc.dram_tensor("xt", [D, M], BF16, kind="ExternalInput")  # pixel^T
    pt = nc.dram_tensor("pt", [D, N], BF16, kind="ExternalInput")  # patch^T
    wq = nc.dram_tensor("wq", [D, C], BF16, kind="ExternalInput")
    wk = nc.dram_tensor("wk", [D, C], BF16, kind="ExternalInput")
    wv = nc.dram_tensor("wv", [D, C], BF16, kind="ExternalInput")
    wo = nc.dram_tensor("wo", [C, D], BF16, kind="ExternalInput")
    yp = nc.dram_tensor("yp", [N, D], F32, kind="ExternalOutput")

    xt_t = xt.ap().rearrange("(kt p) m -> kt p m", p=P)      # [8,128,M]
    pt_t = pt.ap().rearrange("(kt p) n -> kt p n", p=P)
    wq_t = wq.ap().rearrange("(ko ki) c -> ki ko c", ki=P)   # [128,8,256]
    wk_t = wk.ap().rearrange("(ko ki) c -> ki ko c", ki=P)
    wv_t = wv.ap().rearrange("(ko ki) c -> ki ko c", ki=P)
    wo_t = wo.ap().rearrange("(ko ki) n -> ki ko n", ki=P)   # [128,2,1024]
    yp_t = yp.ap().rearrange("(qt p) d -> qt p d", p=P)      # [16,128,1024]

    with tile.TileContext(nc) as tc:
        with tc.tile_pool(name="big", bufs=1) as big:
            wk_sb = big.tile([P, KT_D, C], BF16, name="wk_sb")
            wq_sb = big.tile([P, KT_D, C], BF16, name="wq_sb")
            wv_sb = big.tile([P, KT_D, C], BF16, name="wv_sb")
            wo_sb = big.tile([P, 2, D], BF16, name="wo_sb")
            xt_sb = big.tile([P, KT_D, M], BF16, name="xt_sb")
            pt_sb = big.tile([P, KT_D, N], BF16, name="pt_sb")
            kT = big.tile([P, 2, M], BF16, name="kT")
            qT = big.tile([P, 2, N], BF16, name="qT")
            oT = big.tile([P, 2, N], BF16, name="oT")
            v_sb = big.tile([P, JT, HPC, DH + 1], BF16, name="v_sb")
            warm = big.tile([1, 8], F32, name="warm")
            ones_f = big.tile([P, JT * HPC], F32, name="ones_f")

            # input DMAs; xt first (kT projection is the critical path);
            # wk split in half so the kt=0 matmuls can start sooner
            nc.sync.dma_start(out=wk_sb[:, 0:KT_D // 2, :],
                              in_=wk_t[:, 0:KT_D // 2, :])
            for kt in range(KT_D):
                nc.sync.dma_start(out=xt_sb[:, kt, :], in_=xt_t[kt])
                if kt == 0:
                    nc.sync.dma_start(out=wk_sb[:, KT_D // 2:, :],
                                      in_=wk_t[:, KT_D // 2:, :])
            nc.sync.dma_start(out=wq_sb[:], in_=wq_t)
            for kt in range(KT_D):
                nc.sync.dma_start(out=pt_sb[:, kt, :], in_=pt_t[kt])
            nc.sync.dma_start(out=wv_sb[:], in_=wv_t)
            nc.sync.dma_start(out=wo_sb[:], in_=wo_t)

            # warm the exp table load while DMAs run
            nc.vector.memset(warm[:], 0.0)
            nc.scalar.activation(warm[:], warm[:], EXP, scale=1.0)

            # ones column of V (Z accumulator row)
            nc.vector.memset(ones_f[:], 1.0)
            nc.vector.tensor_copy(
                v_sb[:, :, :, DH],
                ones_f[:].rearrange("p (a b) -> p a b", a=JT))

            # ---- projections kT/qT (kt-outer so matmuls chase the input
            # DMA chunks) and V, all in ONE psum pool: per-bank reuse only,
            # no pool-release serialization between phases ------------------
            with tc.tile_pool(name="prpsum", bufs=1, space="PSUM") as prp:
                def project_T(w_sb, x_sb, out_sb, label):
                    pps = [prp.tile([P, FREE], F32, tag=f"pp{i}",
                                    name=f"pp_{label}{i}")
                           for i in range(8)]
                    for kt in range(KT_D):
                        for mt in range(2):
                            for q4 in range(4):
                                nc.tensor.matmul(
                                    pps[mt * 4 + q4][:],
                                    w_sb[:, kt, mt * P:(mt + 1) * P],
                                    x_sb[:, kt, q4 * FREE:(q4 + 1) * FREE],
                                    start=(kt == 0), stop=(kt == KT_D - 1))
                                if kt == KT_D - 1:
                                    # prompt copy: free the bank right away
                                    # so the next phase's matmuls don't wait
                                    nc.vector.tensor_copy(
                                        out_sb[:, mt,
                                               q4 * FREE:(q4 + 1) * FREE],
                                        pps[mt * 4 + q4][:])

                project_T(wk_sb, xt_sb, kT, "k")
                project_T(wq_sb, pt_sb, qT, "q")

                # V projection: v[m(part), 4h, 64] + ones col
                for st in range(JT):
                    pv = prp.tile([P, C], F32, tag=f"pp{st % 8}", name="pv")
                    for kt in range(KT_D):
                        nc.tensor.matmul(
                            pv[:],
                            xt_sb[:, kt, st * P:(st + 1) * P],
                            wv_sb[:, kt, :],
                            start=(kt == 0), stop=(kt == KT_D - 1))
                    nc.vector.tensor_copy(
                        v_sb[:, st, :, 0:DH],
                        pv[:].rearrange("p (h e) -> p h e", h=HPC))

            # ---- attention + interleaved output projection ----------------
            with (
                tc.tile_pool(name="eT", bufs=3) as epool,
                tc.tile_pool(name="rzp", bufs=4) as rzp,
                tc.tile_pool(name="rzbp", bufs=4) as rzbp,
                tc.tile_pool(name="yout", bufs=3) as yout,
                tc.tile_pool(name="apsum", bufs=1, space="PSUM") as apsum,
            ):
                def yproj_qt(qt, on_act=False):
                    # output projection for one query tile (oT ready);
                    # borrows the "pst" PSUM tag so po can have 4 buffers
                    ysb = yout.tile([P, D], F32, tag="y", name="ysb")
                    py = apsum.tile([P, 2 * FREE], F32, tag="pst",
                                    bufs=2, name="py")
                    for nk in range(2):
                        for ct in range(2):
                            nc.tensor.matmul(
                                py[:, nk * FREE:(nk + 1) * FREE],
                                oT[:, ct, qt * P:(qt + 1) * P],
                                wo_sb[:, ct, nk * FREE:(nk + 1) * FREE],
                                start=(ct == 0), stop=(ct == 1))
                    if on_act:
                        nc.scalar.copy(ysb[:], py[:])
                    else:
                        nc.vector.tensor_copy(ysb[:], py[:])
                    nc.sync.dma_start(out=yp_t[qt], in_=ysb[:])

                for qc in range(QC):
                    for pair in range(2):
                        po = [apsum.tile([DH + 1, FREE], F32, tag="po",
                                         bufs=4, name=f"po{hh}")
                              for hh in range(2)]
                        for jt2 in range(JT // 2):
                            # spread the previous chunk's output projection
                            # through this chunk's slots (its oT deps are
                            # long resolved -> no PE stall), one query tile
                            # at a time so the shared pst tag never starves
                            # the exp pipeline
                            if qc > 0 and (
                                    (pair == 0 and jt2 in (2, 4, 6))
                                    or (pair == 1 and jt2 == 2)):
                                qbase = (qc - 1) * 4
                                off = jt2 // 2 - 1 if pair == 0 else 3
                                yproj_qt(qbase + off)
                            for hh in range(2):
                                pst = apsum.tile([P, 2 * FREE], F32,
                                                 tag="pst", bufs=2,
                                                 name="pst")
                                for k in range(2):
                                    jt = jt2 * 2 + k
                                    nc.tensor.matmul(
                                        pst[:, k * FREE:(k + 1) * FREE],
                                        kT[hh * DH:(hh + 1) * DH, pair,
                                           jt * P:(jt + 1) * P],
                                        qT[hh * DH:(hh + 1) * DH, pair,
                                           qc * FREE:(qc + 1) * FREE],
                                        start=True, stop=True)
                                eT = epool.tile([P, 2 * FREE], BF16,
                                                tag="eT", name="eT")
                                nc.scalar.activation(eT[:], pst[:], EXP,
                                                     scale=SCALE)
                                h = pair * 2 + hh
                                for k in range(2):
                                    jt = jt2 * 2 + k
                                    nc.tensor.matmul(
                                        po[hh][:], v_sb[:, jt, h, :],
                                        eT[:, k * FREE:(k + 1) * FREE],
                                        start=(jt == 0), stop=(jt == JT - 1))
                        for hh in range(2):
                            rz = rzp.tile([1, FREE], F32, tag="rz",
                                          name="rz")
                            nc.vector.reciprocal(rz[:],
                                                 po[hh][DH:DH + 1, :])
                            rzb = rzbp.tile([DH, FREE], F32, tag="rzb",
                                            name="rzb")
                            nc.gpsimd.partition_broadcast(rzb[:], rz[:])
                            nc.vector.tensor_mul(
                                oT[hh * DH:(hh + 1) * DH, pair,
                                   qc * FREE:(qc + 1) * FREE],
                                po[hh][0:DH, :], rzb[:])
                for i, qt in enumerate(range((QC - 1) * 4, QC * 4)):
                    yproj_qt(qt, on_act=(i % 2 == 0))

    nc.compile()
    return nc


def get_nc():
    if "nc" not in _cache:
        _cache["nc"] = _build_nc()
    return _cache["nc"]


def _bf16(x):
    import ml_dtypes
    return np.ascontiguousarray(x, dtype=ml_dtypes.bfloat16)


def make_core_inputs(patch_embed, pixel_embed, Wq, Wk, Wv, Wo, c):
    b, g = divmod(c, HPC)
    sl = slice(g * C, (g + 1) * C)
    return {
        "xt": _bf16(np.asarray(pixel_embed[b], dtype=np.float32).T),
        "pt": _bf16(np.asarray(patch_embed[b], dtype=np.float32).T),
        "wq": _bf16(Wq[:, sl]),
        "wk": _bf16(Wk[:, sl]),
        "wv": _bf16(Wv[:, sl]),
        "wo": _bf16(Wo[sl, :]),
    }


def kernel(patch_embed, pixel_embed, Wq, Wk, Wv, Wo, bo):
    from concourse.bass_utils import run_bass_kernel_spmd

    nc = get_nc()
    in_maps = [make_core_inputs(patch_embed, pixel_embed, Wq, Wk, Wv, Wo, c)
               for c in range(N_CORES)]
    res = run_bass_kernel_spmd(nc, in_maps, core_ids=list(range(N_CORES)))
    out = np.empty((B, N, D), dtype=np.float32)
    for b in range(B):
        acc = res.results[b * HPC + 0]["yp"].astype(np.float32)
        for g in range(1, HPC):
            acc = acc + res.results[b * HPC + g]["yp"]
        out[b] = acc + np.asarray(bo, dtype=np.float32)[None, :]
    return out


# revision 16
# speedup vs baseline: 1.0938x; 1.0938x over previous
"""Cross-attention kernel for Trainium2, 8-core tensor-parallel over heads.

Problem (fixed shapes, fp32 in/out):
    patch_embed [2, 2048, 1024], pixel_embed [2, 2048, 1024]
    Wq/Wk/Wv [1024, 1024], Wo [1024, 1024], bo [1024]
    16 heads x 64 dim_head, softmax cross-attention, out [2, 2048, 1024].

Sharding: core c handles batch b = c // 4 and head-group g = c % 4
(4 heads = 256 inner cols). Each core computes a partial output
(its heads' contribution to out @ Wo); host sums the 4 partials per
batch and adds the bias.

v2 design (bf16 matmuls, host-side transpose):
  - Host pre-transposes patch/pixel to [d, seq] and casts everything to
    bf16, so the device needs no transposes at all (rel err ~6e-3 vs
    the 2e-2 gate; logits stay in fp32 PSUM).
  - Projections run kt-outer across 8 PSUM banks so the matmuls
    pipeline with the input DMA chunks.
  - Attention: per (qc, pair) accumulate po[65, 512] (row 64 = Z via a
    ones-column in V); exp on ACT from fp32 PSUM straight to bf16 SBUF.
  - Normalization (DVE reciprocal + gpsimd broadcast + DVE mul) is
    pipelined behind the PE via 3 po buffers.
  - Output projection for query chunk qc is emitted mid-way through
    attention chunk qc+1 so its matmuls never head-of-line-block the PE.
"""

import numpy as np

HEADS = 16
DH = 64
B = 2
N = 2048          # query seq len
M = 2048          # key seq len
D = 1024
N_CORES = 8
HPC = 4           # heads per core
C = HPC * DH      # 256 inner cols per core
SCALE = DH ** -0.5
P = 128
FREE = 512        # fp32 PSUM bank free dim
KT_D = D // P     # 8 contraction tiles for projections
JT = M // P       # 16 key tiles
QC = N // FREE    # 4 query chunks

_cache = {}


def _build_nc():
    import concourse.bacc as bacc
    import concourse.mybir as mybir
    import concourse.tile as tile

    F32 = mybir.dt.float32
    BF16 = mybir.dt.bfloat16
    EXP = mybir.ActivationFunctionType.Exp

    nc = bacc.Bacc("TRN2", target_bir_lowering=False, debug=False,
                   num_devices=N_CORES)

    xt = nc.dram_tensor("xt", [D, M], BF16, kind="ExternalInput")  # pixel^T
    pt = nc.dram_tensor("pt", [D, N], BF16, kind="ExternalInput")  # patch^T
    wq = nc.dram_tensor("wq", [D, C], BF16, kind="ExternalInput")
    wk = nc.dram_tensor("wk", [D, C], BF16, kind="ExternalInput")
    wv = nc.dram_tensor("wv", [D, C], BF16, kind="ExternalInput")
    wo = nc.dram_tensor("wo", [C, D], BF16, kind="ExternalInput")
    yp = nc.dram_tensor("yp", [N, D], F32, kind="ExternalOutput")

    xt_t = xt.ap().rearrange("(kt p) m -> kt p m", p=P)      # [8,128,M]
    pt_t = pt.ap().rearrange("(kt p) n -> kt p n", p=P)
    wq_t = wq.ap().rearrange("(ko ki) c -> ki ko c", ki=P)   # [128,8,256]
    wk_t = wk.ap().rearrange("(ko ki) c -> ki ko c", ki=P)
    wv_t = wv.ap().rearrange("(ko ki) c -> ki ko c", ki=P)
    wo_t = wo.ap().rearrange("(ko ki) n -> ki ko n", ki=P)   # [128,2,1024]
    yp_t = yp.ap().rearrange("(qt p) d -> qt p d", p=P)      # [16,128,1024]

    with tile.TileContext(nc) as tc:
        with tc.tile_pool(name="big", bufs=1) as big:
            wk_sb = big.tile([P, KT_D, C], BF16, name="wk_sb")
            wq_sb = big.tile([P, KT_D, C], BF16, name="wq_sb")
            wv_sb = big.tile([P, KT_D, C], BF16, name="wv_sb")
            wo_sb = big.tile([P, 2, D], BF16, name="wo_sb")
            xt_sb = big.tile([P, KT_D, M], BF16, name="xt_sb")
            pt_sb = big.tile([P, KT_D, N], BF16, name="pt_sb")
            kT = big.tile([P, 2, M], BF16, name="kT")
            qT = big.tile([P, 2, N], BF16, name="qT")
            oT = big.tile([P, 2, N], BF16, name="oT")
            v_sb = big.tile([P, JT, HPC, DH + 1], BF16, name="v_sb")
            warm = big.tile([1, 8], F32, name="warm")
            ones_f = big.tile([P, JT * HPC], F32, name="ones_f")

            # input DMAs; xt first (kT projection is the critical path);
            # wk split in half so the kt=0 matmuls can start sooner
            nc.sync.dma_start(out=wk_sb[:, 0:KT_D // 2, :],
                              in_=wk_t[:, 0:KT_D // 2, :])
            for kt in range(KT_D):
                nc.sync.dma_start(out=xt_sb[:, kt, :], in_=xt_t[kt])
                if kt == 0:
                    nc.sync.dma_start(out=wk_sb[:, KT_D // 2:, :],
                                      in_=wk_t[:, KT_D // 2:, :])
            nc.sync.dma_start(out=wq_sb[:], in_=wq_t)
            for kt in range(KT_D):
                nc.sync.dma_start(out=pt_sb[:, kt, :], in_=pt_t[kt])
            nc.sync.dma_start(out=wv_sb[:], in_=wv_t)
            nc.sync.dma_start(out=wo_sb[:], in_=wo_t)

            # warm the exp table load while DMAs run
            nc.vector.memset(warm[:], 0.0)
            nc.scalar.activation(warm[:], warm[:], EXP, scale=1.0)

            # ones column of V (Z accumulator row)
            nc.vector.memset(ones_f[:], 1.0)
            nc.vector.tensor_copy(
                v_sb[:, :, :, DH],
                ones_f[:].rearrange("p (a b) -> p a b", a=JT))

            # ---- projections kT/qT (kt-outer so matmuls chase the input
            # DMA chunks) and V, all in ONE psum pool: per-bank reuse only,
            # no pool-release serialization between phases ------------------
            with tc.tile_pool(name="prpsum", bufs=1, space="PSUM") as prp:
                def project_T(w_sb, x_sb, out_sb, label):
                    pps = [prp.tile([P, FREE], F32, tag=f"pp{i}",
                                    name=f"pp_{label}{i}")
                           for i in range(8)]
                    for kt in range(KT_D):
                        for mt in range(2):
                            for q4 in range(4):
                                nc.tensor.matmul(
                                    pps[mt * 4 + q4][:],
                                    w_sb[:, kt, mt * P:(mt + 1) * P],
                                    x_sb[:, kt, q4 * FREE:(q4 + 1) * FREE],
                                    start=(kt == 0), stop=(kt == KT_D - 1))
                                if kt == KT_D - 1:
                                    # prompt copy: free the bank right away
                                    # so the next phase's matmuls don't wait
                                    nc.vector.tensor_copy(
                                        out_sb[:, mt,
                                               q4 * FREE:(q4 + 1) * FREE],
                                        pps[mt * 4 + q4][:])

                project_T(wk_sb, xt_sb, kT, "k")
                project_T(wq_sb, pt_sb, qT, "q")

                # V projection: v[m(part), 4h, 64] + ones col
                for st in range(JT):
                    pv = prp.tile([P, C], F32, tag=f"pp{st % 8}", name="pv")
                    for kt in range(KT_D):
                        nc.tensor.matmul(
                            pv[:],
                            xt_sb[:, kt, st * P:(st + 1) * P],
                            wv_sb[:, kt, :],
                            start=(kt == 0), stop=(kt == KT_D - 1))
                    nc.vector.tensor_copy(
                        v_sb[:, st, :, 0:DH],
                        pv[:].rearrange("p (h e) -> p h e", h=HPC))

            # ---- attention + interleaved output projection ----------------
            with (
                tc.tile_pool(name="eT", bufs=3) as epool,
                tc.tile_pool(name="rzp", bufs=4) as rzp,
                tc.tile_pool(name="rzbp", bufs=4) as rzbp,
                tc.tile_pool(name="yout", bufs=3) as yout,
                tc.tile_pool(name="apsum", bufs=1, space="PSUM") as apsum,
            ):
                def yproj_qt(qt, on_act=False):
                    # output projection for one query tile (oT ready);
                    # borrows the "pst" PSUM tag so po can have 4 buffers
                    ysb = yout.tile([P, D], F32, tag="y", name="ysb")
                    py = apsum.tile([P, 2 * FREE], F32, tag="pst",
                                    bufs=2, name="py")
                    for nk in range(2):
                        for ct in range(2):
                            nc.tensor.matmul(
                                py[:, nk * FREE:(nk + 1) * FREE],
                                oT[:, ct, qt * P:(qt + 1) * P],
                                wo_sb[:, ct, nk * FREE:(nk + 1) * FREE],
                                start=(ct == 0), stop=(ct == 1))
                    if on_act:
                        nc.scalar.copy(ysb[:], py[:])
                    else:
                        nc.vector.tensor_copy(ysb[:], py[:])
                    nc.sync.dma_start(out=yp_t[qt], in_=ysb[:])

                for qc in range(QC):
                    for pair in range(2):
                        po = [apsum.tile([DH + 1, FREE], F32, tag="po",
                                         bufs=4, name=f"po{hh}")
                              for hh in range(2)]
                        for jt2 in range(JT // 2):
                            # spread the previous chunk's output projection
                            # through this chunk's pair-1 slots: far enough
                            # past the boundary that the previous normalize
                            # chain (DVE reciprocals) has fully drained, so
                            # these matmuls never head-of-line-block the PE
                            if qc > 0 and pair == 1 and jt2 in (1, 3, 5, 7):
                                yproj_qt((qc - 1) * 4 + jt2 // 2)
                            for hh in range(2):
                                pst = apsum.tile([P, 2 * FREE], F32,
                                                 tag="pst", bufs=2,
                                                 name="pst")
                                for k in range(2):
                                    jt = jt2 * 2 + k
                                    nc.tensor.matmul(
                                        pst[:, k * FREE:(k + 1) * FREE],
                                        kT[hh * DH:(hh + 1) * DH, pair,
                                           jt * P:(jt + 1) * P],
                                        qT[hh * DH:(hh + 1) * DH, pair,
                                           qc * FREE:(qc + 1) * FREE],
                                        start=True, stop=True)
                                eT = epool.tile([P, 2 * FREE], BF16,
                                                tag="eT", name="eT")
                                nc.scalar.activation(eT[:], pst[:], EXP,
                                                     scale=SCALE)
                                h = pair * 2 + hh
                                for k in range(2):
                                    jt = jt2 * 2 + k
                                    nc.tensor.matmul(
                                        po[hh][:], v_sb[:, jt, h, :],
                                        eT[:, k * FREE:(k + 1) * FREE],
                                        start=(jt == 0), stop=(jt == JT - 1))
                        for hh in range(2):
                            rz = rzp.tile([1, FREE], F32, tag="rz",
                                          name="rz")
                            nc.vector.reciprocal(rz[:],
                                                 po[hh][DH:DH + 1, :])
                            rzb = rzbp.tile([DH, FREE], F32, tag="rzb",
                                            name="rzb")
                            nc.gpsimd.partition_broadcast(rzb[:], rz[:])
                            nc.vector.tensor_mul(
                                oT[hh * DH:(hh + 1) * DH, pair,
                                   qc * FREE:(qc + 1) * FREE],
                                po[hh][0:DH, :], rzb[:])
                for i, qt in enumerate(range((QC - 1) * 4, QC * 4)):
                    yproj_qt(qt, on_act=(i % 2 == 0))

    nc.compile()
    return nc


def get_nc():
    if "nc" not in _cache:
        _cache["nc"] = _build_nc()
    return _cache["nc"]


def _bf16(x):
    import ml_dtypes
    return np.ascontiguousarray(x, dtype=ml_dtypes.bfloat16)


def make_core_inputs(patch_embed, pixel_embed, Wq, Wk, Wv, Wo, c):
    b, g = divmod(c, HPC)
    sl = slice(g * C, (g + 1) * C)
    return {
        "xt": _bf16(np.asarray(pixel_embed[b], dtype=np.float32).T),
        "pt": _bf16(np.asarray(patch_embed[b], dtype=np.float32).T),
        "wq": _bf16(Wq[:, sl]),
        "wk": _bf16(Wk[:, sl]),
        "wv": _bf16(Wv[:, sl]),
        "wo": _bf16(Wo[sl, :]),
    }


def kernel(patch_embed, pixel_embed, Wq, Wk, Wv, Wo, bo):
    from concourse.bass_utils import run_bass_kernel_spmd

    nc = get_nc()
    in_maps = [make_core_inputs(patch_embed, pixel_embed, Wq, Wk, Wv, Wo, c)
               for c in range(N_CORES)]
    res = run_bass_kernel_spmd(nc, in_maps, core_ids=list(range(N_CORES)))
    out = np.empty((B, N, D), dtype=np.float32)
    for b in range(B):
        acc = res.results[b * HPC + 0]["yp"].astype(np.float32)
        for g in range(1, HPC):
            acc = acc + res.results[b * HPC + g]["yp"]
        out[b] = acc + np.asarray(bo, dtype=np.float32)[None, :]
    return out
